# revision 7
# baseline (speedup 1.0000x reference)
"""GRUCell Trainium2 kernel: B=8192, input=hidden=2048, fp32 I/O.

Strategy: data-parallel over batch (1024 rows/core on 8 cores).
Host pre-transposes activations and packs weights so every DMA is
contiguous per partition.

Mixed precision: the r and z gates tolerate fp8 quantization (their
error is damped by the sigmoid slope and the (1-z)n+z*h blend; measured
rel-err ~1.3e-2 vs the 2e-2 gate), so their four GEMMs run as
fp8-e4m3 DoubleRow matmuls (2 k-rows per PE cell). The n gate (tanh,
slope 1) stays fp16. PSUM accumulates fp32 throughout:
out[f, b] = sum_k W[f, k] * act[b, k], stationary = weight tile,
moving = 512 batch columns. The r/z gates accumulate x@W_i.T and
h@W_h.T into the SAME psum bank, so the gate pre-activation comes out
of PSUM ready for one ScalarE sigmoid (bias folded in; the fp8 weight
scale folded into the activation's scale operand). The n gate keeps its
two halves separate (r multiplies only the h half).

_build_bass(reps=N) wraps the whole pass in a hardware For_i loop (same
instructions, executed N times back-to-back) — used by test.py to
measure per-execution device time with the dispatch round-trip
amortized. kernel() itself always uses reps=1.
"""

import numpy as np

B = 8192
H = 2048  # hidden == input size
NCORES = 8
BS = B // NCORES  # 1024 batch rows per core
P = 128
KB = H // P   # 16 contraction blocks
FT = H // P   # 16 feature tiles
NF = 512      # psum free width (one bank of fp32)
NB = BS // NF  # 2 batch halves
KD = KB // 2  # 8 DoubleRow steps (2 k-blocks each)
NW8 = 4 * KB  # fp8 stationary tiles per feature tile (r,z gates)
NW16 = 2 * KB  # fp16 stationary tiles per feature tile (n gate)
WS = 128.0    # fp8 weight scale (power of 2; undone in the ACT scale)

_CACHE = {}


def _build_bass(reps=1):
    import concourse.bacc as bacc
    import concourse.mybir as mybir
    import concourse.tile as tile

    f8 = mybir.dt.float8e4
    f16 = mybir.dt.float16
    f32 = mybir.dt.float32
    AF = mybir.ActivationFunctionType
    DR = mybir.MatmulPerfMode.DoubleRow

    nc = bacc.Bacc(trn_type="TRN2")

    x8 = nc.declare_dram_parameter("x8", [P, KB, BS], f8, isOutput=False)
    h8 = nc.declare_dram_parameter("h8", [P, KB, BS], f8, isOutput=False)
    xT = nc.declare_dram_parameter("xT", [P, KB, BS], f16, isOutput=False)
    hT = nc.declare_dram_parameter("hT", [P, KB, BS], f16, isOutput=False)
    w8 = nc.declare_dram_parameter("w8", [FT, P, NW8, P], f8, isOutput=False)
    w16 = nc.declare_dram_parameter("w16", [FT, P, NW16, P], f16, isOutput=False)
    bpk = nc.declare_dram_parameter("bpk", [P, 4, FT], f32, isOutput=False)
    outT = nc.declare_dram_parameter("outT", [H, BS], f32, isOutput=True)

    with tile.TileContext(nc) as tc:
        with (
            tc.tile_pool(name="res", bufs=1) as res,
            tc.tile_pool(name="wts", bufs=2) as wts,
            tc.tile_pool(name="ew", bufs=2) as ew,
            tc.tile_pool(name="ps", bufs=2, space="PSUM") as ps,
        ):
            bsb = res.tile([P, 4, FT], f32, tag="bsb", bufs=1)
            nc.sync.dma_start(bsb[:], bpk[:])

            # Priming: the ISA leaves room for very few sync-wait commands
            # per compute instruction, so let each engine observe every DMA
            # semaphore it will depend on once, up front. The Sigmoid also
            # absorbs the one-time ACT table load (sigmoid_and_others covers
            # both Sigmoid and Tanh).
            warm = res.tile([P, 1], f32, tag="warm", bufs=1)
            nc.scalar.activation(warm[:], bsb[:, 0, 0:1], AF.Sigmoid)
            warm2 = res.tile([P, 1], f32, tag="warm2", bufs=1)
            nc.vector.tensor_copy(warm2[:], bsb[:, 1, 0:1])

            def body():
                x8sb = res.tile([P, KB, BS], f8, tag="x8sb", bufs=1)
                h8sb = res.tile([P, KB, BS], f8, tag="h8sb", bufs=1)
                xsb = res.tile([P, KB, BS], f16, tag="xsb", bufs=1)
                hsb = res.tile([P, KB, BS], f16, tag="hsb", bufs=1)
                nc.sync.dma_start(x8sb[:], x8[:])
                nc.sync.dma_start(h8sb[:], h8[:])
                nc.sync.dma_start(xsb[:], xT[:])
                nc.sync.dma_start(hsb[:], hT[:])

                for ft in range(FT):
                    w8t = wts.tile([P, NW8, P], f8, tag="w8t", bufs=2)
                    nc.sync.dma_start(w8t[:], w8[ft])
                    w16t = wts.tile([P, NW16, P], f16, tag="w16t", bufs=2)
                    nc.sync.dma_start(w16t[:], w16[ft])
                    for bh in range(NB):
                        ps_r = ps.tile([P, NF], f32, tag="ps_r", bufs=2)
                        ps_z = ps.tile([P, NF], f32, tag="ps_z", bufs=2)
                        ps_ni = ps.tile([P, NF], f32, tag="ps_ni", bufs=2)
                        ps_nh = ps.tile([P, NF], f32, tag="ps_nh", bufs=2)

                        bcol = slice(bh * NF, (bh + 1) * NF)
                        # r/z gates: fp8 DoubleRow, two k-blocks per matmul,
                        # x and h halves accumulated into one psum bank.
                        drs = []
                        for gi, dst in ((0, ps_r), (1, ps_z)):
                            for si, src in ((0, x8sb), (1, h8sb)):
                                goff = (2 * gi + si) * KB
                                for kd in range(KD):
                                    drs.append((
                                        dst, goff + 2 * kd, src, 2 * kd,
                                        si == 0 and kd == 0,
                                        si == 1 and kd == KD - 1,
                                    ))
                        # n gate: fp16, x and h halves kept separate.
                        f16s = []
                        for si, (dst, src) in enumerate(
                            ((ps_ni, xsb), (ps_nh, hsb))
                        ):
                            for kb in range(KB):
                                f16s.append((
                                    dst, si * KB + kb, src, kb,
                                    kb == 0, kb == KB - 1,
                                ))
                        # Interleave fp16 and DoubleRow matmuls 1:1 (pure
                        # emission reorder, same math): the DR stationary
                        # load covers 256 columns (~213ns) and only hides
                        # under a long-enough preceding matmul — a 213ns
                        # fp16 matmul, not a 107ns DR one.
                        for (fdst, fwi, fsrc, fkb, fst, fsp), (
                            ddst, dwi, dsrc, dkd, dst_, dsp,
                        ) in zip(f16s, drs):
                            nc.tensor.matmul(
                                fdst[:],
                                w16t[:, fwi, :],
                                fsrc[:, fkb, bcol],
                                start=fst, stop=fsp,
                                skip_group_check=True,
                            )
                            nc.tensor.matmul(
                                ddst[:],
                                w8t[:, dwi : dwi + 2, :],
                                dsrc[:, dkd : dkd + 2, bcol],
                                start=dst_, stop=dsp,
                                perf_mode=DR,
                                skip_group_check=True,
                            )

                        r = ew.tile([P, NF], f32, tag="r", bufs=2)
                        z = ew.tile([P, NF], f32, tag="z", bufs=2)
                        t = ew.tile([P, NF], f32, tag="t", bufs=2)
                        s = ew.tile([P, NF], f32, tag="s", bufs=2)
                        n = ew.tile([P, NF], f32, tag="n", bufs=2)
                        d = ew.tile([P, NF], f32, tag="d", bufs=2)
                        o = ew.tile([P, NF], f32, tag="o", bufs=3)

                        nc.scalar.activation(
                            r[:], ps_r[:], AF.Sigmoid,
                            bias=bsb[:, 0, ft : ft + 1], scale=1.0 / WS,
                        )
                        nc.scalar.activation(
                            z[:], ps_z[:], AF.Sigmoid,
                            bias=bsb[:, 1, ft : ft + 1], scale=1.0 / WS,
                        )
                        # u = nh + b_hn on ScalarE (Copy w/ bias) so the DVE
                        # mult below has both operands ACT-produced -> a
                        # single cross-engine wait, fitting the crowded 2-src
                        # format.
                        u = ew.tile([P, NF], f32, tag="u", bufs=2)
                        nc.scalar.activation(
                            u[:], ps_nh[:], AF.Identity,
                            bias=bsb[:, 3, ft : ft + 1],
                        )
                        nc.vector.tensor_mul(t[:], u[:], r[:])
                        nc.vector.tensor_add(s[:], ps_ni[:], t[:])
                        nc.scalar.activation(
                            n[:], s[:], AF.Tanh, bias=bsb[:, 2, ft : ft + 1]
                        )
                        # h_new = n + z*(h - n)
                        nc.vector.tensor_sub(d[:], hsb[:, ft, bcol], n[:])
                        nc.vector.tensor_mul(d[:], z[:], d[:])
                        nc.vector.tensor_add(o[:], n[:], d[:])
                        nc.sync.dma_start(
                            outT[ft * P : (ft + 1) * P, bcol], o[:]
                        )

            if reps == 1:
                body()
            else:
                # staggered_reset: back-edge jumps straight to the body and
                # semaphore resets are staged through it, instead of a
                # full-stop all-engine barrier between iterations.
                with tc.For_i(0, reps, staggered_reset=True):
                    body()
    nc.compile()
    return nc


def _pack_act(a, dtype):
    # [p, kb, b_global]: element = a[b, kb*128+p]
    return np.ascontiguousarray(
        a.T.astype(dtype).reshape(KB, P, B).transpose(1, 0, 2)
    )


def _prep_inputs(inputs):
    import ml_dtypes

    f8 = ml_dtypes.float8_e4m3  # TRN FP8_EXP4 (bias 7, max 240)
    x = np.asarray(inputs["x"], np.float32)
    h = np.asarray(inputs["h"], np.float32)
    xT = _pack_act(x, np.float16)
    hT = _pack_act(h, np.float16)
    x8 = _pack_act(x, f8)
    h8 = _pack_act(h, f8)

    w8 = np.empty([FT, P, NW8, P], f8)
    for g, key in enumerate(["W_ir", "W_hr", "W_iz", "W_hz"]):
        WT = (np.asarray(inputs[key], np.float32).T * WS).astype(f8)  # [k, f]
        t = WT.reshape(KB, P, FT, P)  # [kb, k_in, ft, f_in]
        w8[:, :, g * KB : (g + 1) * KB, :] = t.transpose(2, 1, 0, 3)

    w16 = np.empty([FT, P, NW16, P], np.float16)
    for g, key in enumerate(["W_in", "W_hn"]):
        WT = np.asarray(inputs[key], np.float32).T.astype(np.float16)
        t = WT.reshape(KB, P, FT, P)
        w16[:, :, g * KB : (g + 1) * KB, :] = t.transpose(2, 1, 0, 3)

    b_r = inputs["b_ir"] + inputs["b_hr"]
    b_z = inputs["b_iz"] + inputs["b_hz"]
    bpk = np.stack([b_r, b_z, inputs["b_in"], inputs["b_hn"]]).astype(np.float32)
    # [4, 2048] -> [p, 4, ft]: element = bias_g[ft*128+p]
    bpk = np.ascontiguousarray(bpk.reshape(4, FT, P).transpose(2, 0, 1))

    in_maps = []
    for c in range(NCORES):
        cols = slice(c * BS, (c + 1) * BS)
        in_maps.append(
            {
                "x8": np.ascontiguousarray(x8[:, :, cols]),
                "h8": np.ascontiguousarray(h8[:, :, cols]),
                "xT": np.ascontiguousarray(xT[:, :, cols]),
                "hT": np.ascontiguousarray(hT[:, :, cols]),
                "w8": w8,
                "w16": w16,
                "bpk": bpk,
            }
        )
    return in_maps


def kernel(**inputs):
    from concourse.bass_utils import run_bass_kernel_spmd

    if "nc" not in _CACHE:
        _CACHE["nc"] = _build_bass()
    nc = _CACHE["nc"]
    in_maps = _prep_inputs(inputs)
    res = run_bass_kernel_spmd(nc, in_maps, list(range(NCORES))).results
    outT = np.concatenate([res[c]["outT"] for c in range(NCORES)], axis=1)
    return np.ascontiguousarray(outT.T).astype(np.float32)


# revision 8
# speedup vs baseline: 1.1757x; 1.1757x over previous
"""GRUCell Trainium2 kernel: B=8192, input=hidden=2048, fp32 I/O.

Strategy: data-parallel over batch (1024 rows/core on 8 cores).
Host pre-transposes activations and packs weights so every DMA is
contiguous per partition.

Mixed precision: the r and z gates tolerate fp8 quantization (their
error is damped by the sigmoid slope and the (1-z)n+z*h blend; measured
rel-err ~1.3e-2 vs the 2e-2 gate), so their four GEMMs run as
fp8-e4m3 DoubleRow matmuls (2 k-rows per PE cell). The n gate (tanh,
slope 1) stays fp16. PSUM accumulates fp32 throughout:
out[f, b] = sum_k W[f, k] * act[b, k], stationary = weight tile,
moving = 512 batch columns. The r/z gates accumulate x@W_i.T and
h@W_h.T into the SAME psum bank, so the gate pre-activation comes out
of PSUM ready for one ScalarE sigmoid (bias folded in; the fp8 weight
scale folded into the activation's scale operand). The n gate keeps its
two halves separate (r multiplies only the h half).

_build_bass(reps=N) wraps the whole pass in a hardware For_i loop (same
instructions, executed N times back-to-back) — used by test.py to
measure per-execution device time with the dispatch round-trip
amortized. kernel() itself always uses reps=1.
"""

import numpy as np

B = 8192
H = 2048  # hidden == input size
NCORES = 8
BS = B // NCORES  # 1024 batch rows per core
P = 128
KB = H // P   # 16 contraction blocks
FT = H // P   # 16 feature tiles
NF = 512      # psum free width (one bank of fp32)
NB = BS // NF  # 2 batch halves
KD = KB // 2  # 8 DoubleRow steps (2 k-blocks each)
NW8 = 4 * KB  # fp8 stationary tiles per feature tile (r,z gates)
NW16 = 2 * KB  # fp16 stationary tiles per feature tile (n gate)
WS = 128.0    # fp8 weight scale (power of 2; undone in the ACT scale)

_CACHE = {}


def _build_bass(reps=1):
    import concourse.bacc as bacc
    import concourse.mybir as mybir
    import concourse.tile as tile

    f8 = mybir.dt.float8e4
    f16 = mybir.dt.float16
    f32 = mybir.dt.float32
    AF = mybir.ActivationFunctionType
    DR = mybir.MatmulPerfMode.DoubleRow

    nc = bacc.Bacc(trn_type="TRN2")

    x8 = nc.declare_dram_parameter("x8", [P, KB, BS], f8, isOutput=False)
    h8 = nc.declare_dram_parameter("h8", [P, KB, BS], f8, isOutput=False)
    xT = nc.declare_dram_parameter("xT", [P, KB, BS], f16, isOutput=False)
    hT = nc.declare_dram_parameter("hT", [P, KB, BS], f16, isOutput=False)
    w8 = nc.declare_dram_parameter("w8", [FT, P, NW8, P], f8, isOutput=False)
    w16 = nc.declare_dram_parameter("w16", [FT, P, NW16, P], f16, isOutput=False)
    bpk = nc.declare_dram_parameter("bpk", [P, 4, FT], f32, isOutput=False)
    outT = nc.declare_dram_parameter("outT", [H, BS], f32, isOutput=True)

    with tile.TileContext(nc) as tc:
        with (
            tc.tile_pool(name="res", bufs=1) as res,
            tc.tile_pool(name="wts", bufs=2) as wts,
            tc.tile_pool(name="ew", bufs=2) as ew,
            tc.tile_pool(name="ps", bufs=2, space="PSUM") as ps,
        ):
            bsb = res.tile([P, 4, FT], f32, tag="bsb", bufs=1)
            nc.sync.dma_start(bsb[:], bpk[:])

            # Priming: the ISA leaves room for very few sync-wait commands
            # per compute instruction, so let each engine observe every DMA
            # semaphore it will depend on once, up front. The Sigmoid also
            # absorbs the one-time ACT table load (sigmoid_and_others covers
            # both Sigmoid and Tanh).
            warm = res.tile([P, 1], f32, tag="warm", bufs=1)
            nc.scalar.activation(warm[:], bsb[:, 0, 0:1], AF.Sigmoid)
            warm2 = res.tile([P, 1], f32, tag="warm2", bufs=1)
            nc.vector.tensor_copy(warm2[:], bsb[:, 1, 0:1])

            def body():
                x8sb = res.tile([P, KB, BS], f8, tag="x8sb", bufs=1)
                h8sb = res.tile([P, KB, BS], f8, tag="h8sb", bufs=1)
                xsb = res.tile([P, KB, BS], f16, tag="xsb", bufs=1)
                hsb = res.tile([P, KB, BS], f16, tag="hsb", bufs=1)
                nc.sync.dma_start(x8sb[:], x8[:])
                nc.sync.dma_start(h8sb[:], h8[:])
                nc.sync.dma_start(xsb[:], xT[:])
                nc.sync.dma_start(hsb[:], hT[:])

                for ft in range(FT):
                    w8t = wts.tile([P, NW8, P], f8, tag="w8t", bufs=2)
                    nc.sync.dma_start(w8t[:], w8[ft])
                    w16t = wts.tile([P, NW16, P], f16, tag="w16t", bufs=2)
                    nc.sync.dma_start(w16t[:], w16[ft])
                    for bh in range(NB):
                        ps_r = ps.tile([P, NF], f32, tag="ps_r", bufs=2)
                        ps_z = ps.tile([P, NF], f32, tag="ps_z", bufs=2)
                        ps_ni = ps.tile([P, NF], f32, tag="ps_ni", bufs=2)
                        ps_nh = ps.tile([P, NF], f32, tag="ps_nh", bufs=2)

                        bcol = slice(bh * NF, (bh + 1) * NF)
                        # r/z gates: fp8 DoubleRow, two k-blocks per matmul,
                        # x and h halves accumulated into one psum bank.
                        drs = []
                        for gi, dst in ((0, ps_r), (1, ps_z)):
                            for si, src in ((0, x8sb), (1, h8sb)):
                                goff = (2 * gi + si) * KB
                                for kd in range(KD):
                                    drs.append((
                                        dst, goff + 2 * kd, src, 2 * kd,
                                        si == 0 and kd == 0,
                                        si == 1 and kd == KD - 1,
                                    ))
                        # n gate: fp16, x and h halves kept separate.
                        f16s = []
                        for si, (dst, src) in enumerate(
                            ((ps_ni, xsb), (ps_nh, hsb))
                        ):
                            for kb in range(KB):
                                f16s.append((
                                    dst, si * KB + kb, src, kb,
                                    kb == 0, kb == KB - 1,
                                ))
                        # Interleave fp16 and DoubleRow matmuls 1:1 (pure
                        # emission reorder, same math): the DR stationary
                        # load covers 256 columns (~213ns) and only hides
                        # under a long-enough preceding matmul — a 213ns
                        # fp16 matmul, not a 107ns DR one.
                        for (fdst, fwi, fsrc, fkb, fst, fsp), (
                            ddst, dwi, dsrc, dkd, dst_, dsp,
                        ) in zip(f16s, drs):
                            nc.tensor.matmul(
                                fdst[:],
                                w16t[:, fwi, :],
                                fsrc[:, fkb, bcol],
                                start=fst, stop=fsp,
                                skip_group_check=True,
                            )
                            nc.tensor.matmul(
                                ddst[:],
                                w8t[:, dwi : dwi + 2, :],
                                dsrc[:, dkd : dkd + 2, bcol],
                                start=dst_, stop=dsp,
                                perf_mode=DR,
                                skip_group_check=True,
                            )

                        r = ew.tile([P, NF], f32, tag="r", bufs=2)
                        z = ew.tile([P, NF], f32, tag="z", bufs=2)
                        t = ew.tile([P, NF], f32, tag="t", bufs=2)
                        s = ew.tile([P, NF], f32, tag="s", bufs=2)
                        n = ew.tile([P, NF], f32, tag="n", bufs=2)
                        d = ew.tile([P, NF], f32, tag="d", bufs=2)
                        o = ew.tile([P, NF], f32, tag="o", bufs=3)

                        nc.scalar.activation(
                            r[:], ps_r[:], AF.Sigmoid,
                            bias=bsb[:, 0, ft : ft + 1], scale=1.0 / WS,
                        )
                        nc.scalar.activation(
                            z[:], ps_z[:], AF.Sigmoid,
                            bias=bsb[:, 1, ft : ft + 1], scale=1.0 / WS,
                        )
                        # u = nh + b_hn on ScalarE (Copy w/ bias) so the DVE
                        # mult below has both operands ACT-produced -> a
                        # single cross-engine wait, fitting the crowded 2-src
                        # format.
                        u = ew.tile([P, NF], f32, tag="u", bufs=2)
                        nc.scalar.activation(
                            u[:], ps_nh[:], AF.Identity,
                            bias=bsb[:, 3, ft : ft + 1],
                        )
                        nc.vector.tensor_mul(t[:], u[:], r[:])
                        nc.vector.tensor_add(s[:], ps_ni[:], t[:])
                        nc.scalar.activation(
                            n[:], s[:], AF.Tanh, bias=bsb[:, 2, ft : ft + 1]
                        )
                        # h_new = n + z*(h - n)
                        nc.vector.tensor_sub(d[:], hsb[:, ft, bcol], n[:])
                        nc.vector.tensor_mul(d[:], z[:], d[:])
                        nc.vector.tensor_add(o[:], n[:], d[:])
                        nc.sync.dma_start(
                            outT[ft * P : (ft + 1) * P, bcol], o[:]
                        )

            if reps == 1:
                body()
            else:
                with tc.For_i(0, reps):
                    body()
    nc.compile()
    return nc


def _pack_act(a, dtype):
    # [p, kb, b_global]: element = a[b, kb*128+p]
    return np.ascontiguousarray(
        a.T.astype(dtype).reshape(KB, P, B).transpose(1, 0, 2)
    )


def _prep_inputs(inputs):
    import ml_dtypes

    f8 = ml_dtypes.float8_e4m3  # TRN FP8_EXP4 (bias 7, max 240)
    x = np.asarray(inputs["x"], np.float32)
    h = np.asarray(inputs["h"], np.float32)
    xT = _pack_act(x, np.float16)
    hT = _pack_act(h, np.float16)
    x8 = _pack_act(x, f8)
    h8 = _pack_act(h, f8)

    w8 = np.empty([FT, P, NW8, P], f8)
    for g, key in enumerate(["W_ir", "W_hr", "W_iz", "W_hz"]):
        WT = (np.asarray(inputs[key], np.float32).T * WS).astype(f8)  # [k, f]
        t = WT.reshape(KB, P, FT, P)  # [kb, k_in, ft, f_in]
        w8[:, :, g * KB : (g + 1) * KB, :] = t.transpose(2, 1, 0, 3)

    w16 = np.empty([FT, P, NW16, P], np.float16)
    for g, key in enumerate(["W_in", "W_hn"]):
        WT = np.asarray(inputs[key], np.float32).T.astype(np.float16)
        t = WT.reshape(KB, P, FT, P)
        w16[:, :, g * KB : (g + 1) * KB, :] = t.transpose(2, 1, 0, 3)

    b_r = inputs["b_ir"] + inputs["b_hr"]
    b_z = inputs["b_iz"] + inputs["b_hz"]
    bpk = np.stack([b_r, b_z, inputs["b_in"], inputs["b_hn"]]).astype(np.float32)
    # [4, 2048] -> [p, 4, ft]: element = bias_g[ft*128+p]
    bpk = np.ascontiguousarray(bpk.reshape(4, FT, P).transpose(2, 0, 1))

    in_maps = []
    for c in range(NCORES):
        cols = slice(c * BS, (c + 1) * BS)
        in_maps.append(
            {
                "x8": np.ascontiguousarray(x8[:, :, cols]),
                "h8": np.ascontiguousarray(h8[:, :, cols]),
                "xT": np.ascontiguousarray(xT[:, :, cols]),
                "hT": np.ascontiguousarray(hT[:, :, cols]),
                "w8": w8,
                "w16": w16,
                "bpk": bpk,
            }
        )
    return in_maps


def kernel(**inputs):
    from concourse.bass_utils import run_bass_kernel_spmd

    if "nc" not in _CACHE:
        _CACHE["nc"] = _build_bass()
    nc = _CACHE["nc"]
    in_maps = _prep_inputs(inputs)
    res = run_bass_kernel_spmd(nc, in_maps, list(range(NCORES))).results
    outT = np.concatenate([res[c]["outT"] for c in range(NCORES)], axis=1)
    return np.ascontiguousarray(outT.T).astype(np.float32)
